# revision 22
# baseline (speedup 1.0000x reference)
"""MoE grouped-experts (SwiGLU) kernel for Trainium2, expert-parallel over 8 cores.

Problem: T=8192 tokens, top_k=2, E=8 experts, DIM=2048, HIDDEN=1408.
Routing is balanced: slot i = (token i//2, k i%2) -> expert i % 8, so expert
pair (2p, 2p+1) both process exactly the tokens t with t % 4 == p.

Sharding (expert-parallel per the hint): core e holds expert e's weights and
computes out_e = (silu(xg @ w1_e^T) * (xg @ w3_e^T)) @ w2_e^T * score for its
2048 routed tokens. Host does the dispatch (strided slice of x, transposed and
cast to bf16) and the combine (pairwise add + row interleave).

Device layout per core (all matmuls bf16 in / fp32 psum accumulate):
  xgT    [2048(dim), 2048(tok)] bf16   - routed tokens, transposed
  w1T/w3T[2048(dim), 1408(hid)] bf16   - resident in SBUF
  w2T    [1408(hid), 2048(dim)] bf16   - resident in SBUF
  scores [128, 16] fp32                - scores[p, tt] = score(token tt*128+p)
  out    [2048(tok), 2048(dim)] bf16   - scaled partial output

All DMAs are issued in consumption order so the PE never waits on HBM:
the preamble interleaves xg-chunk0 tiles with w1 d-tiles, and chunk 0's
GEMM1 runs dd-major, accumulating h-tiles 0..7 in all 8 PSUM banks, so
matmuls start as soon as the first (xg0[dd], w1[dd]) pair lands (~12us:
SPMD entry barrier ~7us + ~0.6us/DMA-trigger + first-tile transfer) and
stay paced with the DMA stream. w3/w2/scores/xg1..3 stream in behind and
are resident long before first use; later chunks never touch HBM except
the xg prefetch and out writes.

GEMM1/3: psum[h=128, tok=512] += w1T[d,h].T @ xgT[d, tok-chunk]  (16 d-tiles)
h = silu(psum1) * psum3 -> sbuf bf16 [h, tok]
GEMM2:   psum[tok=128, d=512] += h[h, tok].T @ w2T[h, d-chunk]   (11 h-tiles)
out = psum * score[token]  (per-partition scalar on ACT engine)
"""

import os
import sys
from contextlib import ExitStack

import numpy as np

try:
    import concourse.bass as bass
except ImportError:  # pragma: no cover
    sys.path.insert(0, "/opt/trn_rl_repo")
    import concourse.bass as bass

import ml_dtypes

import concourse.tile as tile
from concourse import mybir
from concourse.bass_utils import run_bass_kernel_spmd

T, TOPK, E = 8192, 2, 8
DIM, HID = 2048, 1408
TOK = (T * TOPK) // E        # 2048 tokens (slots) per expert/core
CHUNK = 512                  # token chunk for GEMM1/3 moving dim
NCH = TOK // CHUNK           # 4
DT = DIM // 128              # 16 contraction tiles for GEMM1/3
HT = HID // 128              # 11 contraction tiles for GEMM2
DC = DIM // 512              # 4 output-dim chunks for GEMM2
TTC = CHUNK // 128           # 4 token tiles per chunk
NTT = TOK // 128             # 16 token tiles total
NB1 = 8                      # psum banks used by the chunk-0 ramp (5 pg + 3 po)

_BF = mybir.dt.bfloat16
_F32 = mybir.dt.float32
_bf16 = ml_dtypes.bfloat16


def _build_bass():
    nc = bass.Bass("TRN2", target_bir_lowering=False, debug=False)
    xgT = nc.declare_dram_parameter("xgT", [DIM, TOK], _BF, isOutput=False).ap()
    w1t = nc.declare_dram_parameter("w1T", [DIM, HID], _BF, isOutput=False).ap()
    w3t = nc.declare_dram_parameter("w3T", [DIM, HID], _BF, isOutput=False).ap()
    w2t = nc.declare_dram_parameter("w2T", [HID, DIM], _BF, isOutput=False).ap()
    sc = nc.declare_dram_parameter("scores", [128, NTT], _F32, isOutput=False).ap()
    out = nc.declare_dram_parameter("out", [TOK, DIM], _BF, isOutput=True).ap()

    with tile.TileContext(nc) as tc, ExitStack() as ctx:
        wp = ctx.enter_context(tc.tile_pool(name="w", bufs=1))
        xp = ctx.enter_context(tc.tile_pool(name="xg", bufs=17))
        hp = ctx.enter_context(tc.tile_pool(name="h", bufs=2))
        sp = ctx.enter_context(tc.tile_pool(name="sil", bufs=12))
        cp = ctx.enter_context(tc.tile_pool(name="p3c", bufs=2))
        op = ctx.enter_context(tc.tile_pool(name="ost", bufs=6))
        pg = ctx.enter_context(tc.tile_pool(name="pg", bufs=5, space="PSUM"))
        po = ctx.enter_context(tc.tile_pool(name="po", bufs=3, space="PSUM"))

        # Resident weights: w1T/w3T as 16 [128(d),1408(h)] tiles side by side,
        # w2T as 11 [128(h),2048(d)] tiles side by side. All fit in SBUF
        # together (~132 KB/partition), so nothing streams during GEMM2.
        w1s = wp.tile([128, DT * HID], _BF, tag="w1")
        w3s = wp.tile([128, DT * HID], _BF, tag="w3")
        w2s = wp.tile([128, HT * DIM], _BF, tag="w2")
        scs = wp.tile([128, NTT], _F32, tag="sc")

        def w1sl(dd, hh):
            return w1s[:, dd * HID + hh * 128: dd * HID + hh * 128 + 128]

        def w3sl(dd, hh):
            return w3s[:, dd * HID + hh * 128: dd * HID + hh * 128 + 128]

        def w2sl(hh, dc):
            return w2s[:, hh * DIM + dc * 512: hh * DIM + dc * 512 + 512]

        # ---- DMA preamble, strictly in consumption order ----
        # The 16 DMA engines are SHARED between the Sync and Scalar HWDGE
        # queues, so all ramp-critical input traffic stays on the Sync
        # queue in exact consumption order (mixing queues splits per-engine
        # bandwidth and slows every tile). chunk-0 xg tiles interleave with
        # w1 d-tiles: each ~0.5 MB pair feeds 8 ramp matmuls (~1.7us of PE
        # work per ~1.2us of DMA).
        # w1[0] alone is column-split: the first matmul only needs its
        # hh 0..7 columns (the ramp's NB1 partial banks), so the critical
        # first transfer shrinks from 491KB to 387KB. One extra trigger;
        # the deferred tail lands long before phase B reads hh 8..10.
        # dd=0's tiles are additionally split into partition-halves: a single
        # DMA only engages ~4 of the 16 engines, so two half-tile DMAs cut
        # the critical first-transfer latency roughly in half.
        xg = {}
        CA = NB1 * 128
        for dd in range(DT):
            t = xp.tile([128, CHUNK], _BF, tag="xg")
            if dd == 0:
                for p0 in (0, 64):
                    nc.sync.dma_start(t[p0:p0 + 64, :],
                                      xgT[p0:p0 + 64, 0:CHUNK])
                    nc.sync.dma_start(
                        w1s[p0:p0 + 64, 0:CA], w1t[p0:p0 + 64, 0:CA])
            else:
                nc.sync.dma_start(t[:], xgT[dd * 128:(dd + 1) * 128, 0:CHUNK])
                nc.sync.dma_start(w1s[:, dd * HID:(dd + 1) * HID],
                                  w1t[dd * 128:(dd + 1) * 128, :])
            xg[dd] = t
        nc.sync.dma_start(w1s[:, CA:HID], w1t[0:128, CA:HID])
        for dd in range(DT):
            nc.sync.dma_start(w3s[:, dd * HID:(dd + 1) * HID],
                              w3t[dd * 128:(dd + 1) * 128, :])
        for hh in range(HT):
            nc.sync.dma_start(w2s[:, hh * DIM:(hh + 1) * DIM],
                              w2t[hh * 128:(hh + 1) * 128, :])
        nc.sync.dma_start(scs[:], sc[:])

        sil = {}

        def gemm3_and_h(hs, hh, p3):
            # TT insts have one sync-wait slot; extra waits are hoisted by
            # _split_multi_waits, but routing the psum drain through ACT
            # keeps the DVE multiply single-wait in the common case.
            p3c = cp.tile([128, CHUNK], _BF, tag="p3c")
            nc.scalar.copy(p3c[:], p3[:])
            nc.vector.tensor_mul(hs[:, hh * CHUNK:(hh + 1) * CHUNK],
                                 sil[hh][:], p3c[:])

        def gemm2(ch, hs):
            for dc in range(DC):
                for tt in range(TTC):
                    gtt = ch * TTC + tt
                    pot = po.tile([128, 512], _F32, tag="po")
                    for hh in range(HT):
                        nc.tensor.matmul(
                            pot[:],
                            hs[:, hh * CHUNK + tt * 128:
                               hh * CHUNK + tt * 128 + 128],
                            w2sl(hh, dc),
                            start=(hh == 0), stop=(hh == HT - 1))
                    ost = op.tile([128, 512], _BF, tag="ost")
                    nc.scalar.mul(ost[:], pot[:], scs[:, gtt:gtt + 1])
                    # trigger the out write from the Scalar queue: it sits
                    # right behind its producing scale (wait pre-satisfied)
                    # and keeps Sync free for the xg prefetch stream.
                    nc.scalar.dma_start(
                        out[gtt * 128:(gtt + 1) * 128,
                            dc * 512:(dc + 1) * 512], ost[:])

        def prefetch_xg(ch):
            for dd in range(DT):
                t = xp.tile([128, CHUNK], _BF, tag="xg")
                nc.sync.dma_start(
                    t[:], xgT[dd * 128:(dd + 1) * 128,
                              ch * CHUNK:(ch + 1) * CHUNK])
                xg[dd] = t

        # ---- chunk 0: dd-major ramp so the PE starts as DMAs land ----
        # GEMM1 for h-tiles 0..7 accumulates in all 8 psum banks; matmul
        # (dd, hh) only needs xg0[dd] + w1[dd], which arrive ~1.2us apart.
        p1s = [pg.tile([128, CHUNK], _F32, tag="pg", name=f"p1r{i}")
               for i in range(5)] + \
              [po.tile([128, CHUNK], _F32, tag="po", name=f"p1r{5 + i}")
               for i in range(3)]
        for dd in range(DT):
            for hh in range(NB1):
                nc.tensor.matmul(p1s[hh][:], w1sl(dd, hh), xg[dd][:],
                                 start=(dd == 0), stop=(dd == DT - 1))
        for hh in range(NB1):
            s = sp.tile([128, CHUNK], _BF, tag="sil")
            nc.scalar.activation(s[:], p1s[hh][:],
                                 mybir.ActivationFunctionType.Silu)
            sil[hh] = s
        for hh in range(NB1, HT):
            p1 = pg.tile([128, CHUNK], _F32, tag="pg")
            for dd in range(DT):
                nc.tensor.matmul(p1[:], w1sl(dd, hh), xg[dd][:],
                                 start=(dd == 0), stop=(dd == DT - 1))
            s = sp.tile([128, CHUNK], _BF, tag="sil")
            nc.scalar.activation(s[:], p1[:],
                                 mybir.ActivationFunctionType.Silu)
            sil[hh] = s
        hs0 = hp.tile([128, HT * CHUNK], _BF, tag="h")
        for hh in range(HT):
            p3 = pg.tile([128, CHUNK], _F32, tag="pg")
            for dd in range(DT):
                nc.tensor.matmul(p3[:], w3sl(dd, hh), xg[dd][:],
                                 start=(dd == 0), stop=(dd == DT - 1))
            gemm3_and_h(hs0, hh, p3)
        prefetch_xg(1)
        gemm2(0, hs0)

        # ---- chunks 1..3: everything resident, standard interleaved form ----
        for ch in range(1, NCH):
            hs = hp.tile([128, HT * CHUNK], _BF, tag="h")
            for hh in range(HT):
                p1 = pg.tile([128, CHUNK], _F32, tag="pg")
                p3 = pg.tile([128, CHUNK], _F32, tag="pg")
                for dd in range(DT):
                    nc.tensor.matmul(p1[:], w1sl(dd, hh), xg[dd][:],
                                     start=(dd == 0), stop=(dd == DT - 1))
                for dd in range(DT):
                    nc.tensor.matmul(p3[:], w3sl(dd, hh), xg[dd][:],
                                     start=(dd == 0), stop=(dd == DT - 1))
                s = sp.tile([128, CHUNK], _BF, tag="sil")
                nc.scalar.activation(s[:], p1[:],
                                     mybir.ActivationFunctionType.Silu)
                sil[hh] = s
                gemm3_and_h(hs, hh, p3)
            if ch < NCH - 1:
                prefetch_xg(ch + 1)
            gemm2(ch, hs)
    _split_multi_waits(nc)
    return nc


def _split_multi_waits(nc):
    """TPB compute instructions have a single sync-wait slot; walrus codegen
    rejects more. Hoist all-but-one wait into standalone EventSemaphore
    instructions on the same (in-order) engine queue right before."""
    n = 0
    for fn in nc.m.functions:
        for bb in fn.blocks:
            out_list = []
            for inst in bb.instructions:
                si = inst.sync_info
                if si is not None and si.on_wait and len(si.on_wait) > 1:
                    while len(si.on_wait) > 1:
                        w = si.on_wait.pop(0)
                        ev = mybir.InstEventSemaphore(
                            name=f"hoistw_{n}", ins=[], outs=[])
                        n += 1
                        ev.engine = inst.engine
                        ev.sync_info = mybir.SyncInfo(on_wait=[w], on_update=[])
                        out_list.append(ev)
                out_list.append(inst)
            bb.instructions[:] = out_list
    return n


_NC_CACHE = None


def _get_nc():
    global _NC_CACHE
    if _NC_CACHE is None:
        _NC_CACHE = _build_bass()
    return _NC_CACHE


def _expected_indices():
    return (np.arange(T * TOPK, dtype=np.int64) % E).reshape(T, TOPK)


def _make_in_maps(x, top_scores, selected_experts_indices, w1, w2, w3):
    """Host-side dispatch: build the 8 per-core input dicts.

    Returns (in_maps, combine) where combine(partials) -> full [T, DIM] fp32.
    """
    fast = np.array_equal(selected_experts_indices, _expected_indices())
    in_maps = []
    if fast:
        # expert e takes tokens t = e//2 + 4j, score column e % 2
        xg_cache = {}
        for e in range(E):
            p = e // 2
            if p not in xg_cache:
                xg_cache[p] = np.ascontiguousarray(
                    x[p::4].astype(_bf16).T)          # [DIM, TOK] bf16
            s = top_scores[p::4, e % 2].astype(np.float32)        # [TOK]
            in_maps.append({
                "xgT": xg_cache[p],
                "w1T": np.ascontiguousarray(w1[e].astype(_bf16).T),
                "w3T": np.ascontiguousarray(w3[e].astype(_bf16).T),
                "w2T": np.ascontiguousarray(w2[e].astype(_bf16).T),
                "scores": np.ascontiguousarray(s.reshape(NTT, 128).T),
            })

        def combine(partials):
            outf = np.empty((T, DIM), np.float32)
            for p in range(4):
                outf[p::4] = partials[2 * p] + partials[2 * p + 1]
            return outf

        return in_maps, combine

    # General balanced-routing fallback: stable-sort dispatch on host.
    flat_expert = selected_experts_indices.reshape(-1)
    perm = np.argsort(flat_expert, kind="stable")
    counts = np.bincount(flat_expert, minlength=E)
    assert (counts == TOK).all(), f"unbalanced routing: {counts}"
    src_token = perm // TOPK
    flat_scores = top_scores.reshape(-1)[perm].astype(np.float32)
    for e in range(E):
        sl = slice(e * TOK, (e + 1) * TOK)
        xg = x[src_token[sl]]                                     # [TOK, DIM]
        s = flat_scores[sl]
        in_maps.append({
            "xgT": np.ascontiguousarray(xg.astype(_bf16).T),
            "w1T": np.ascontiguousarray(w1[e].astype(_bf16).T),
            "w3T": np.ascontiguousarray(w3[e].astype(_bf16).T),
            "w2T": np.ascontiguousarray(w2[e].astype(_bf16).T),
            "scores": np.ascontiguousarray(s.reshape(NTT, 128).T),
        })

    def combine(partials):
        outf = np.zeros((T, DIM), np.float32)
        for e in range(E):
            sl = slice(e * TOK, (e + 1) * TOK)
            np.add.at(outf, src_token[sl], partials[e])
        return outf

    return in_maps, combine


def _run(inputs, trace=False, trace_cores=None, tmpdir=None):
    x = np.asarray(inputs["x"], np.float32)
    top_scores = np.asarray(inputs["top_scores"], np.float32)
    sel = np.asarray(inputs["selected_experts_indices"])
    w1 = np.asarray(inputs["w1"], np.float32)
    w2 = np.asarray(inputs["w2"], np.float32)
    w3 = np.asarray(inputs["w3"], np.float32)
    in_maps, combine = _make_in_maps(x, top_scores, sel, w1, w2, w3)
    nc = _get_nc()
    res = run_bass_kernel_spmd(
        nc, in_maps, list(range(E)), trace=trace,
        trace_cores=trace_cores, tmpdir=tmpdir)
    partials = [np.asarray(r["out"], np.float32) for r in res.results]
    return combine(partials), res


def kernel(**inputs) -> np.ndarray:
    out, _ = _run(inputs, trace=False)
    return out


# revision 23
# speedup vs baseline: 1.0009x; 1.0009x over previous
"""MoE grouped-experts (SwiGLU) kernel for Trainium2, expert-parallel over 8 cores.

Problem: T=8192 tokens, top_k=2, E=8 experts, DIM=2048, HIDDEN=1408.
Routing is balanced: slot i = (token i//2, k i%2) -> expert i % 8, so expert
pair (2p, 2p+1) both process exactly the tokens t with t % 4 == p.

Sharding (expert-parallel per the hint): core e holds expert e's weights and
computes out_e = (silu(xg @ w1_e^T) * (xg @ w3_e^T)) @ w2_e^T * score for its
2048 routed tokens. Host does the dispatch (strided slice of x, transposed and
cast to bf16) and the combine (pairwise add + row interleave).

Device layout per core (all matmuls bf16 in / fp32 psum accumulate):
  xgT    [2048(dim), 2048(tok)] bf16   - routed tokens, transposed
  w1T/w3T[2048(dim), 1408(hid)] bf16   - resident in SBUF
  w2T    [1408(hid), 2048(dim)] bf16   - resident in SBUF
  scores [128, 16] fp32                - scores[p, tt] = score(token tt*128+p)
  out    [2048(tok), 2048(dim)] bf16   - scaled partial output

All DMAs are issued in consumption order so the PE never waits on HBM:
the preamble interleaves xg-chunk0 tiles with w1 d-tiles, and chunk 0's
GEMM1 runs dd-major, accumulating h-tiles 0..7 in all 8 PSUM banks, so
matmuls start as soon as the first (xg0[dd], w1[dd]) pair lands (~12us:
SPMD entry barrier ~7us + ~0.6us/DMA-trigger + first-tile transfer) and
stay paced with the DMA stream. w3/w2/scores/xg1..3 stream in behind and
are resident long before first use; later chunks never touch HBM except
the xg prefetch and out writes.

GEMM1/3: psum[h=128, tok=512] += w1T[d,h].T @ xgT[d, tok-chunk]  (16 d-tiles)
h = silu(psum1) * psum3 -> sbuf bf16 [h, tok]
GEMM2:   psum[tok=128, d=512] += h[h, tok].T @ w2T[h, d-chunk]   (11 h-tiles)
out = psum * score[token]  (per-partition scalar on ACT engine)
"""

import os
import sys
from contextlib import ExitStack

import numpy as np

try:
    import concourse.bass as bass
except ImportError:  # pragma: no cover
    sys.path.insert(0, "/opt/trn_rl_repo")
    import concourse.bass as bass

import ml_dtypes

import concourse.tile as tile
from concourse import mybir
from concourse.bass_utils import run_bass_kernel_spmd

T, TOPK, E = 8192, 2, 8
DIM, HID = 2048, 1408
TOK = (T * TOPK) // E        # 2048 tokens (slots) per expert/core
CHUNK = 512                  # token chunk for GEMM1/3 moving dim
NCH = TOK // CHUNK           # 4
DT = DIM // 128              # 16 contraction tiles for GEMM1/3
HT = HID // 128              # 11 contraction tiles for GEMM2
DC = DIM // 512              # 4 output-dim chunks for GEMM2
TTC = CHUNK // 128           # 4 token tiles per chunk
NTT = TOK // 128             # 16 token tiles total
NB1 = 8                      # psum banks used by the chunk-0 ramp (5 pg + 3 po)

_BF = mybir.dt.bfloat16
_F32 = mybir.dt.float32
_bf16 = ml_dtypes.bfloat16


def _build_bass():
    nc = bass.Bass("TRN2", target_bir_lowering=False, debug=False)
    xgT = nc.declare_dram_parameter("xgT", [DIM, TOK], _BF, isOutput=False).ap()
    w1t = nc.declare_dram_parameter("w1T", [DIM, HID], _BF, isOutput=False).ap()
    w3t = nc.declare_dram_parameter("w3T", [DIM, HID], _BF, isOutput=False).ap()
    w2t = nc.declare_dram_parameter("w2T", [HID, DIM], _BF, isOutput=False).ap()
    sc = nc.declare_dram_parameter("scores", [128, NTT], _F32, isOutput=False).ap()
    out = nc.declare_dram_parameter("out", [TOK, DIM], _BF, isOutput=True).ap()

    with tile.TileContext(nc) as tc, ExitStack() as ctx:
        wp = ctx.enter_context(tc.tile_pool(name="w", bufs=1))
        xp = ctx.enter_context(tc.tile_pool(name="xg", bufs=17))
        hp = ctx.enter_context(tc.tile_pool(name="h", bufs=2))
        sp = ctx.enter_context(tc.tile_pool(name="sil", bufs=12))
        cp = ctx.enter_context(tc.tile_pool(name="p3c", bufs=2))
        op = ctx.enter_context(tc.tile_pool(name="ost", bufs=6))
        pg = ctx.enter_context(tc.tile_pool(name="pg", bufs=5, space="PSUM"))
        po = ctx.enter_context(tc.tile_pool(name="po", bufs=3, space="PSUM"))

        # Resident weights: w1T/w3T as 16 [128(d),1408(h)] tiles side by side,
        # w2T as 11 [128(h),2048(d)] tiles side by side. All fit in SBUF
        # together (~132 KB/partition), so nothing streams during GEMM2.
        w1s = wp.tile([128, DT * HID], _BF, tag="w1")
        w3s = wp.tile([128, DT * HID], _BF, tag="w3")
        w2s = wp.tile([128, HT * DIM], _BF, tag="w2")
        scs = wp.tile([128, NTT], _F32, tag="sc")

        def w1sl(dd, hh):
            return w1s[:, dd * HID + hh * 128: dd * HID + hh * 128 + 128]

        def w3sl(dd, hh):
            return w3s[:, dd * HID + hh * 128: dd * HID + hh * 128 + 128]

        def w2sl(hh, dc):
            return w2s[:, hh * DIM + dc * 512: hh * DIM + dc * 512 + 512]

        # ---- DMA preamble, strictly in consumption order ----
        # The 16 DMA engines are SHARED between the Sync and Scalar HWDGE
        # queues, so all ramp-critical input traffic stays on the Sync
        # queue in exact consumption order (mixing queues splits per-engine
        # bandwidth and slows every tile). chunk-0 xg tiles interleave with
        # w1 d-tiles: each ~0.5 MB pair feeds 8 ramp matmuls (~1.7us of PE
        # work per ~1.2us of DMA).
        # w1[0] alone is column-split: the first matmul only needs its
        # hh 0..7 columns (the ramp's NB1 partial banks), so the critical
        # first transfer shrinks from 491KB to 387KB. One extra trigger;
        # the deferred tail lands long before phase B reads hh 8..10.
        xg = {}
        CA = NB1 * 128
        for dd in range(DT):
            t = xp.tile([128, CHUNK], _BF, tag="xg")
            nc.sync.dma_start(t[:], xgT[dd * 128:(dd + 1) * 128, 0:CHUNK])
            xg[dd] = t
            cw = CA if dd == 0 else HID
            nc.sync.dma_start(w1s[:, dd * HID: dd * HID + cw],
                              w1t[dd * 128:(dd + 1) * 128, 0:cw])
        nc.sync.dma_start(w1s[:, CA:HID], w1t[0:128, CA:HID])
        for dd in range(DT):
            nc.sync.dma_start(w3s[:, dd * HID:(dd + 1) * HID],
                              w3t[dd * 128:(dd + 1) * 128, :])
        for hh in range(HT):
            nc.sync.dma_start(w2s[:, hh * DIM:(hh + 1) * DIM],
                              w2t[hh * 128:(hh + 1) * 128, :])
        nc.sync.dma_start(scs[:], sc[:])

        sil = {}

        def gemm3_and_h(hs, hh, p3):
            # TT insts have one sync-wait slot; extra waits are hoisted by
            # _split_multi_waits, but routing the psum drain through ACT
            # keeps the DVE multiply single-wait in the common case.
            p3c = cp.tile([128, CHUNK], _BF, tag="p3c")
            nc.scalar.copy(p3c[:], p3[:])
            nc.vector.tensor_mul(hs[:, hh * CHUNK:(hh + 1) * CHUNK],
                                 sil[hh][:], p3c[:])

        def gemm2(ch, hs):
            for dc in range(DC):
                for tt in range(TTC):
                    gtt = ch * TTC + tt
                    pot = po.tile([128, 512], _F32, tag="po")
                    for hh in range(HT):
                        nc.tensor.matmul(
                            pot[:],
                            hs[:, hh * CHUNK + tt * 128:
                               hh * CHUNK + tt * 128 + 128],
                            w2sl(hh, dc),
                            start=(hh == 0), stop=(hh == HT - 1))
                    ost = op.tile([128, 512], _BF, tag="ost")
                    nc.scalar.mul(ost[:], pot[:], scs[:, gtt:gtt + 1])
                    # trigger the out write from the Scalar queue: it sits
                    # right behind its producing scale (wait pre-satisfied)
                    # and keeps Sync free for the xg prefetch stream.
                    nc.scalar.dma_start(
                        out[gtt * 128:(gtt + 1) * 128,
                            dc * 512:(dc + 1) * 512], ost[:])

        def prefetch_xg(ch):
            for dd in range(DT):
                t = xp.tile([128, CHUNK], _BF, tag="xg")
                nc.sync.dma_start(
                    t[:], xgT[dd * 128:(dd + 1) * 128,
                              ch * CHUNK:(ch + 1) * CHUNK])
                xg[dd] = t

        # ---- chunk 0: dd-major ramp so the PE starts as DMAs land ----
        # GEMM1 for h-tiles 0..7 accumulates in all 8 psum banks; matmul
        # (dd, hh) only needs xg0[dd] + w1[dd], which arrive ~1.2us apart.
        p1s = [pg.tile([128, CHUNK], _F32, tag="pg", name=f"p1r{i}")
               for i in range(5)] + \
              [po.tile([128, CHUNK], _F32, tag="po", name=f"p1r{5 + i}")
               for i in range(3)]
        for dd in range(DT):
            for hh in range(NB1):
                nc.tensor.matmul(p1s[hh][:], w1sl(dd, hh), xg[dd][:],
                                 start=(dd == 0), stop=(dd == DT - 1))
        for hh in range(NB1):
            s = sp.tile([128, CHUNK], _BF, tag="sil")
            nc.scalar.activation(s[:], p1s[hh][:],
                                 mybir.ActivationFunctionType.Silu)
            sil[hh] = s
        for hh in range(NB1, HT):
            p1 = pg.tile([128, CHUNK], _F32, tag="pg")
            for dd in range(DT):
                nc.tensor.matmul(p1[:], w1sl(dd, hh), xg[dd][:],
                                 start=(dd == 0), stop=(dd == DT - 1))
            s = sp.tile([128, CHUNK], _BF, tag="sil")
            nc.scalar.activation(s[:], p1[:],
                                 mybir.ActivationFunctionType.Silu)
            sil[hh] = s
        hs0 = hp.tile([128, HT * CHUNK], _BF, tag="h")
        for hh in range(HT):
            p3 = pg.tile([128, CHUNK], _F32, tag="pg")
            for dd in range(DT):
                nc.tensor.matmul(p3[:], w3sl(dd, hh), xg[dd][:],
                                 start=(dd == 0), stop=(dd == DT - 1))
            gemm3_and_h(hs0, hh, p3)
        prefetch_xg(1)
        gemm2(0, hs0)

        # ---- chunks 1..3: everything resident, standard interleaved form ----
        for ch in range(1, NCH):
            hs = hp.tile([128, HT * CHUNK], _BF, tag="h")
            for hh in range(HT):
                p1 = pg.tile([128, CHUNK], _F32, tag="pg")
                p3 = pg.tile([128, CHUNK], _F32, tag="pg")
                for dd in range(DT):
                    nc.tensor.matmul(p1[:], w1sl(dd, hh), xg[dd][:],
                                     start=(dd == 0), stop=(dd == DT - 1))
                for dd in range(DT):
                    nc.tensor.matmul(p3[:], w3sl(dd, hh), xg[dd][:],
                                     start=(dd == 0), stop=(dd == DT - 1))
                s = sp.tile([128, CHUNK], _BF, tag="sil")
                nc.scalar.activation(s[:], p1[:],
                                     mybir.ActivationFunctionType.Silu)
                sil[hh] = s
                gemm3_and_h(hs, hh, p3)
            if ch < NCH - 1:
                prefetch_xg(ch + 1)
            gemm2(ch, hs)
    _split_multi_waits(nc)
    return nc


def _split_multi_waits(nc):
    """TPB compute instructions have a single sync-wait slot; walrus codegen
    rejects more. Hoist all-but-one wait into standalone EventSemaphore
    instructions on the same (in-order) engine queue right before."""
    n = 0
    for fn in nc.m.functions:
        for bb in fn.blocks:
            out_list = []
            for inst in bb.instructions:
                si = inst.sync_info
                if si is not None and si.on_wait and len(si.on_wait) > 1:
                    while len(si.on_wait) > 1:
                        w = si.on_wait.pop(0)
                        ev = mybir.InstEventSemaphore(
                            name=f"hoistw_{n}", ins=[], outs=[])
                        n += 1
                        ev.engine = inst.engine
                        ev.sync_info = mybir.SyncInfo(on_wait=[w], on_update=[])
                        out_list.append(ev)
                out_list.append(inst)
            bb.instructions[:] = out_list
    return n


_NC_CACHE = None


def _get_nc():
    global _NC_CACHE
    if _NC_CACHE is None:
        _NC_CACHE = _build_bass()
    return _NC_CACHE


def _expected_indices():
    return (np.arange(T * TOPK, dtype=np.int64) % E).reshape(T, TOPK)


def _make_in_maps(x, top_scores, selected_experts_indices, w1, w2, w3):
    """Host-side dispatch: build the 8 per-core input dicts.

    Returns (in_maps, combine) where combine(partials) -> full [T, DIM] fp32.
    """
    fast = np.array_equal(selected_experts_indices, _expected_indices())
    in_maps = []
    if fast:
        # expert e takes tokens t = e//2 + 4j, score column e % 2
        xg_cache = {}
        for e in range(E):
            p = e // 2
            if p not in xg_cache:
                xg_cache[p] = np.ascontiguousarray(
                    x[p::4].astype(_bf16).T)          # [DIM, TOK] bf16
            s = top_scores[p::4, e % 2].astype(np.float32)        # [TOK]
            in_maps.append({
                "xgT": xg_cache[p],
                "w1T": np.ascontiguousarray(w1[e].astype(_bf16).T),
                "w3T": np.ascontiguousarray(w3[e].astype(_bf16).T),
                "w2T": np.ascontiguousarray(w2[e].astype(_bf16).T),
                "scores": np.ascontiguousarray(s.reshape(NTT, 128).T),
            })

        def combine(partials):
            outf = np.empty((T, DIM), np.float32)
            for p in range(4):
                outf[p::4] = partials[2 * p] + partials[2 * p + 1]
            return outf

        return in_maps, combine

    # General balanced-routing fallback: stable-sort dispatch on host.
    flat_expert = selected_experts_indices.reshape(-1)
    perm = np.argsort(flat_expert, kind="stable")
    counts = np.bincount(flat_expert, minlength=E)
    assert (counts == TOK).all(), f"unbalanced routing: {counts}"
    src_token = perm // TOPK
    flat_scores = top_scores.reshape(-1)[perm].astype(np.float32)
    for e in range(E):
        sl = slice(e * TOK, (e + 1) * TOK)
        xg = x[src_token[sl]]                                     # [TOK, DIM]
        s = flat_scores[sl]
        in_maps.append({
            "xgT": np.ascontiguousarray(xg.astype(_bf16).T),
            "w1T": np.ascontiguousarray(w1[e].astype(_bf16).T),
            "w3T": np.ascontiguousarray(w3[e].astype(_bf16).T),
            "w2T": np.ascontiguousarray(w2[e].astype(_bf16).T),
            "scores": np.ascontiguousarray(s.reshape(NTT, 128).T),
        })

    def combine(partials):
        outf = np.zeros((T, DIM), np.float32)
        for e in range(E):
            sl = slice(e * TOK, (e + 1) * TOK)
            np.add.at(outf, src_token[sl], partials[e])
        return outf

    return in_maps, combine


def _run(inputs, trace=False, trace_cores=None, tmpdir=None):
    x = np.asarray(inputs["x"], np.float32)
    top_scores = np.asarray(inputs["top_scores"], np.float32)
    sel = np.asarray(inputs["selected_experts_indices"])
    w1 = np.asarray(inputs["w1"], np.float32)
    w2 = np.asarray(inputs["w2"], np.float32)
    w3 = np.asarray(inputs["w3"], np.float32)
    in_maps, combine = _make_in_maps(x, top_scores, sel, w1, w2, w3)
    nc = _get_nc()
    res = run_bass_kernel_spmd(
        nc, in_maps, list(range(E)), trace=trace,
        trace_cores=trace_cores, tmpdir=tmpdir)
    partials = [np.asarray(r["out"], np.float32) for r in res.results]
    return combine(partials), res


def kernel(**inputs) -> np.ndarray:
    out, _ = _run(inputs, trace=False)
    return out
